# revision 14
# baseline (speedup 1.0000x reference)
"""Trainium2 Bass kernel for nn_MultiHeadLocalAttention_1683627180144.

Full-input contract: kernel(**inputs) takes the complete unsharded inputs and
returns the complete output.  Internally the work is sharded 8 ways
(batch x sequence-half, with a one-window halo) and run SPMD on 8 NeuronCores
via run_bass_kernel_spmd.  Per-core differences (positions for RoPE, edge
masks) are carried entirely by the per-core input data so every core runs the
same program.

Per-core dataflow (2176 tokens = 128-token halo + 2048 queries):
  Stage A (per 128-token tile): LayerNorm (token-major, affine folded into
    weights on host) -> xn bf16; PE transposes xn into xnT (dim-major);
    v projection token-major into per-head slabs with an appended ones column
    (bias added via a ones-row matmul); gate projection head-major.
  Stage B (interleaved): q/k projection chunk pairs stream through a small
    SBUF pool; RoPE per chunk (partition-swap via small DMAs + 3 DVE muls);
    per head pair, query-window-major local attention: sim^T[key, query] in
    256-col [back|own] slots -> exp (Scalar) -> 0/1 mask mul (DVE) -> AV with
    the ones column yielding the softmax denominator in row 64 -> Scalar copy
    to SBUF -> approx-reciprocal * sigmoid-gate -> scale broadcast via DRAM
    bounce -> y^T slabs bf16.
  Phase 3: out = y^T.T @ w_out per token tile, Scalar copy, DMA to DRAM.
"""

import os
import sys

import numpy as np

for _p in ("/opt/trn_rl_repo", "/opt/pypackages"):
    if os.path.isdir(_p) and _p not in sys.path:
        sys.path.append(_p)

import ml_dtypes  # noqa: E402

import concourse.bass as bass  # noqa: E402
import concourse.bacc as bacc  # noqa: E402
import concourse.mybir as mybir  # noqa: E402
from concourse.tile import TileContext  # noqa: E402

BF16 = ml_dtypes.bfloat16

# Problem constants (hardcoded per spec).
B, N, DIM = 4, 4096, 1024
H, DH, WS = 16, 64, 128
NCORES = 8
P = 128
T = 2176          # tokens per shard incl halo window
NT = T // P       # 17 token windows
NQ = 2048         # query tokens per shard
KB = DIM // P     # 8 contraction chunks
QSCALE = DH ** -0.5

f32 = mybir.dt.float32
bf16 = mybir.dt.bfloat16

_PROGRAM_CACHE = {}


def _build_nc():
    """Build the per-core Bass program (same program on all 8 cores)."""
    nc = bacc.Bacc("TRN2")

    x_d = nc.declare_dram_parameter("x", [T, DIM], f32, isOutput=False)
    wqk_d = nc.declare_dram_parameter("wqk", [DIM, 2 * DIM], bf16, isOutput=False)
    wv_d = nc.declare_dram_parameter("wv", [DIM, DIM], bf16, isOutput=False)
    wg_d = nc.declare_dram_parameter("wg", [DIM, H], bf16, isOutput=False)
    wo_d = nc.declare_dram_parameter("wo", [DIM, DIM], bf16, isOutput=False)
    bqk_d = nc.declare_dram_parameter("bqk", [2 * DIM, 1], f32, isOutput=False)
    bv_d = nc.declare_dram_parameter("bv", [1, DIM], bf16, isOutput=False)
    bg_d = nc.declare_dram_parameter("bg", [H, 1], f32, isOutput=False)
    cos_d = nc.declare_dram_parameter("cos", [P, T], bf16, isOutput=False)
    sin_d = nc.declare_dram_parameter("sin", [P, T], bf16, isOutput=False)
    m0_d = nc.declare_dram_parameter("mask0", [P, 1024], bf16, isOutput=False)
    mu_d = nc.declare_dram_parameter("masku", [P, 1024], bf16, isOutput=False)
    out_d = nc.declare_dram_parameter("out", [NQ, DIM], f32, isOutput=True)
    # internal DRAM bounce buffer for the per-query scale broadcast
    scr_d = nc.dram_tensor("scalescr", [2 * H, 1024], bf16)

    with TileContext(nc) as tc:
        from contextlib import ExitStack

        with ExitStack() as ctx:
            consts = ctx.enter_context(tc.tile_pool(name="consts", bufs=1))
            persist = ctx.enter_context(tc.tile_pool(name="persist", bufs=1))

            # Constants.
            mask0 = consts.tile([P, 1024], bf16, tag="mask0")
            masku = consts.tile([P, 1024], bf16, tag="masku")
            nc.sync.dma_start(mask0, m0_d[:, :])
            nc.sync.dma_start(masku, mu_d[:, :])
            bg_t = consts.tile([H, 1], f32, tag="bg")
            nc.sync.dma_start(bg_t, bg_d[:, :])
            eps_t = consts.tile([P, 1], f32, tag="eps")
            nc.vector.memset(eps_t, 1e-5)
            bqk_t = consts.tile([P, 16], f32, tag="bqk")
            nc.sync.dma_start(bqk_t, bqk_d[:, :].rearrange("(m p) o -> p (m o)", p=P))
            bv_t = consts.tile([1, DIM], bf16, tag="bv")
            nc.sync.dma_start(bv_t, bv_d[:, :])
            ones1 = consts.tile([1, P], bf16, tag="ones1")
            nc.vector.memset(ones1, 1.0)
            ident = consts.tile([P, P], bf16, tag="ident")
            from concourse.masks import make_identity
            make_identity(nc, ident)
            cos_t = consts.tile([P, T], bf16, tag="cos")
            sin_t = consts.tile([P, T], bf16, tag="sin")
            nc.sync.dma_start(cos_t, cos_d[:, :])
            nc.sync.dma_start(sin_t, sin_d[:, :])

            # Persistent state.
            # xnT: dim-major normalized input; chunk k occupies cols [k*T, (k+1)*T).
            xnT = persist.tile([P, KB * T], bf16, tag="xnT", name="xnT")
            vsl = [persist.tile([P, H * 65], bf16, tag=f"v{i}", name=f"v{i}")
                   for i in range(NT)]
            gates_s = persist.tile([H, T], bf16, tag="gates_s")
            gsq = persist.tile([8, 2 * H * P], bf16, tag="gsq")
            y = [persist.tile([P, NQ], bf16, tag=f"y{i}", name=f"y{i}")
                 for i in range(KB)]

            # ---------------- Stage A: LN, PE transpose, v & gates ----------
            with ExitStack() as stA:
                p_x = stA.enter_context(tc.tile_pool(name="p_x", bufs=3))
                p_st = stA.enter_context(tc.tile_pool(name="p_st", bufs=6))
                p_xn = stA.enter_context(tc.tile_pool(name="p_xn", bufs=3))
                p_wres = stA.enter_context(tc.tile_pool(name="p_wres", bufs=1))
                p_graw = stA.enter_context(tc.tile_pool(name="p_graw", bufs=1))
                p_pt = stA.enter_context(
                    tc.tile_pool(name="p_pt", bufs=3, space="PSUM"))
                p_psv = stA.enter_context(
                    tc.tile_pool(name="p_psv", bufs=2, space="PSUM"))
                p_psg = stA.enter_context(
                    tc.tile_pool(name="p_psg", bufs=2, space="PSUM"))

                wv_sb = [p_wres.tile([P, DIM], bf16, tag=f"wv{k}", name=f"wv{k}")
                         for k in range(KB)]
                wg_sb = [p_wres.tile([P, H], bf16, tag=f"wg{k}", name=f"wg{k}")
                         for k in range(KB)]
                for k in range(KB):
                    nc.sync.dma_start(wv_sb[k], wv_d[k * P: (k + 1) * P, :])
                    nc.sync.dma_start(wg_sb[k], wg_d[k * P: (k + 1) * P, :])

                graw = p_graw.tile([H, T], f32, tag="graw")

                # gate spans end at these tiles
                gspans = {3: (0, 512), 7: (512, 1024), 11: (1024, 1536),
                          15: (1536, 2048), 16: (2048, 2176)}

                for mt in range(NT):
                    x_t = p_x.tile([P, DIM], f32, tag="x")
                    nc.gpsimd.dma_start(x_t, x_d[mt * P: (mt + 1) * P, :])
                    st = p_st.tile([P, 2, 6], f32, tag="st")
                    nc.vector.bn_stats(st[:, 0, :], x_t[:, 0:512])
                    nc.vector.bn_stats(st[:, 1, :], x_t[:, 512:1024])
                    mv = p_st.tile([P, 2], f32, tag="mv")
                    nc.vector.bn_aggr(mv, st)
                    rs = p_st.tile([P, 1], f32, tag="rs")
                    nc.scalar.activation(
                        rs, mv[:, 1:2], mybir.ActivationFunctionType.Sqrt,
                        bias=eps_t, scale=1.0,
                    )
                    nc.vector.reciprocal(rs, rs)
                    xn_t = p_xn.tile([P, DIM], bf16, tag="xn")
                    nc.vector.tensor_scalar(
                        out=xn_t, in0=x_t,
                        scalar1=mv[:, 0:1], scalar2=rs,
                        op0=mybir.AluOpType.subtract, op1=mybir.AluOpType.mult,
                    )
                    # PE transpose: xn [tok, dim] -> xnT chunks [dim, tok].
                    pt = p_pt.tile([P, KB * P], bf16, tag="pt")
                    for k in range(KB):
                        nc.tensor.transpose(
                            pt[:, k * P: (k + 1) * P],
                            xn_t[:, k * P: (k + 1) * P],
                            ident,
                        )
                    # single strided copy into the 8 chunk slots
                    nc.scalar.copy(
                        xnT.rearrange("p (kb t) -> p kb t", kb=KB)[
                            :, :, mt * P: (mt + 1) * P],
                        pt.rearrange("p (kb c) -> p kb c", kb=KB),
                    )

                    # v projection for this tile
                    nc.vector.memset(
                        vsl[mt].rearrange("p (h c) -> p h c", c=65)[:, :, 64:65],
                        1.0,
                    )
                    for half in range(2):
                        ps = p_psv.tile([P, 512], f32, tag="psv")
                        for k in range(KB):
                            nc.tensor.matmul(
                                ps,
                                xnT[:, k * T + mt * P: k * T + (mt + 1) * P],
                                wv_sb[k][:, half * 512: (half + 1) * 512],
                                start=(k == 0),
                                stop=False,
                            )
                        # bias add via ones-row matmul
                        nc.tensor.matmul(
                            ps,
                            ones1,
                            bv_t[:, half * 512: (half + 1) * 512],
                            start=False,
                            stop=True,
                        )
                        nc.scalar.copy(
                            vsl[mt].rearrange("p (h c) -> p h c", c=65)[
                                :, half * 8: (half + 1) * 8, 0:64],
                            ps.rearrange("p (h c) -> p h c", c=64),
                        )

                    if mt in gspans:
                        n0, n1 = gspans[mt]
                        psg = p_psg.tile([H, 512], f32, tag="psg")
                        for k in range(KB):
                            nc.tensor.matmul(
                                psg[:, : n1 - n0],
                                wg_sb[k],
                                xnT[:, k * T + n0: k * T + n1],
                                start=(k == 0),
                                stop=(k == KB - 1),
                            )
                        nc.scalar.activation(
                            graw[:, n0:n1], psg[:, : n1 - n0],
                            mybir.ActivationFunctionType.Identity,
                            bias=bg_t,
                        )

                # one sigmoid pass over all gates (single act-table flip)
                nc.scalar.activation(
                    gates_s, graw, mybir.ActivationFunctionType.Sigmoid)

            # squish gates into [8, 128] blocks per (head, half-of-queries)
            for h in range(H):
                for hv in range(2):
                    nc.gpsimd.dma_start(
                        gsq[:, (2 * h + hv) * P: (2 * h + hv + 1) * P],
                        gates_s[h: h + 1, P + 1024 * hv: P + 1024 * (hv + 1)],
                    )

            # ---------------- Stage B: qk proj + RoPE + attention -----------
            with ExitStack() as stB:
                p_qk = stB.enter_context(tc.tile_pool(name="p_qk", bufs=2))
                p_w = stB.enter_context(tc.tile_pool(name="p_w", bufs=2))
                p_rope = stB.enter_context(tc.tile_pool(name="p_rope", bufs=2))
                p_em = stB.enter_context(tc.tile_pool(name="p_em", bufs=4))
                p_un = stB.enter_context(tc.tile_pool(name="p_un", bufs=2))
                p_sc = stB.enter_context(tc.tile_pool(name="p_sc", bufs=2))
                p_scb = stB.enter_context(tc.tile_pool(name="p_scb", bufs=2))
                p_wo = stB.enter_context(tc.tile_pool(name="p_wo", bufs=1))
                p_ps2 = stB.enter_context(
                    tc.tile_pool(name="p_ps2", bufs=2, space="PSUM"))
                p_pav = stB.enter_context(
                    tc.tile_pool(name="p_pav", bufs=2, space="PSUM"))

                wo_sb = [p_wo.tile([P, DIM], bf16, tag=f"wo{k}", name=f"wo{k}")
                         for k in range(KB)]

                qk_tiles = {}

                def emit_proj(c, parts=(0, 1)):
                    """Project q chunk c (parts 0) and/or k chunk 8+c (part 1)
                    into the pair tile, then RoPE in place."""
                    if 0 in parts:
                        pair = p_qk.tile([P, 2, T], bf16, tag="qkpair",
                                         name=f"qkpair{c}")
                        qk_tiles[c] = pair
                    else:
                        pair = qk_tiles[c]
                    for which, ch in ((0, c), (1, 8 + c)):
                        if which not in parts:
                            continue
                        w_t = p_w.tile([P, KB, P], bf16, tag="wqk")
                        nc.sync.dma_start(
                            w_t,
                            wqk_d[:, ch * P: (ch + 1) * P].rearrange(
                                "(kb p) c -> p kb c", p=P),
                        )
                        spans = ([(P, 640), (640, 1152), (1152, 1664), (1664, T)]
                                 if which == 0 else
                                 [(0, 512), (512, 1024), (1024, 1536),
                                  (1536, 2048), (2048, T)])
                        for (n0, n1) in spans:
                            ps = p_ps2.tile([P, 1024], f32, tag="ps2")
                            for k in range(KB):
                                nc.tensor.matmul(
                                    ps[:, : n1 - n0],
                                    w_t[:, k, :],
                                    xnT[:, k * T + n0: k * T + n1],
                                    start=(k == 0),
                                    stop=(k == KB - 1),
                                )
                            nc.scalar.activation(
                                pair[:, which, n0:n1], ps[:, : n1 - n0],
                                mybir.ActivationFunctionType.Identity,
                                bias=bqk_t[:, ch: ch + 1],
                            )
                        # RoPE in place (q chunks: skip never-read halo cols).
                        r0 = P if which == 0 else 0
                        rot = p_rope.tile([P, T], bf16, tag="rot")
                        for blk in range(4):
                            src = (blk // 2) * 64 + ((blk + 1) % 2) * 32
                            nc.sync.dma_start(
                                rot[blk * 32: (blk + 1) * 32, r0:],
                                pair[src: src + 32, which, r0:],
                            )
                        qc = p_rope.tile([P, T], bf16, tag="rot", name="qc")
                        nc.vector.tensor_mul(
                            qc[:, r0:], pair[:, which, r0:], cos_t[:, r0:])
                        nc.vector.tensor_mul(
                            rot[:, r0:], rot[:, r0:], sin_t[:, r0:])
                        nc.vector.tensor_add(
                            pair[:, which, r0:], qc[:, r0:], rot[:, r0:])

                def emit_qk(h, hv):
                    """QK sim + exp + mask for one (head, query-half) unit."""
                    pair = qk_tiles[h // 2]
                    po = 64 * (h % 2)
                    qh = pair[po: po + 64, 0, :]
                    kh = pair[po: po + 64, 1, :]
                    # sim tiles A (query windows 1-4) and B (5-8), slots of
                    # 256 cols = [back(128) | own(128)] per query window.
                    ems = []
                    for g in range(2):
                        ps = p_ps2.tile([P, 1024], f32, tag="ps2", name="sim")
                        for wi in range(4):
                            w = 4 * g + wi + 1          # query window in half
                            kw = 8 * hv + w             # own key window
                            qs = (8 * hv + w) * P       # query cols
                            nc.tensor.matmul(
                                ps[:, 256 * wi: 256 * wi + 128],
                                kh[:, (kw - 1) * P: kw * P],
                                qh[:, qs: qs + P],
                                start=True, stop=True,
                            )
                            nc.tensor.matmul(
                                ps[:, 256 * wi + 128: 256 * wi + 256],
                                kh[:, kw * P: (kw + 1) * P],
                                qh[:, qs: qs + P],
                                start=True, stop=True,
                            )
                        em = p_em.tile([P, 1024], bf16, tag="em", name="em")
                        nc.scalar.activation(
                            em, ps, mybir.ActivationFunctionType.Exp)
                        mk = mask0 if (g == 0 and hv == 0) else masku
                        nc.vector.tensor_mul(em, em, mk)
                        ems.append(em)
                    return ems

                def emit_av(h, hv, ems):
                    po = 64 * (h % 2)
                    pav = p_pav.tile([65, 1024], f32, tag="pav", name="pav")
                    for w in range(1, 9):
                        g, wi = (w - 1) // 4, (w - 1) % 4
                        kw = 8 * hv + w
                        cols = slice((w - 1) * P, w * P)
                        nc.tensor.matmul(
                            pav[:, cols],
                            vsl[kw - 1][:, h * 65: (h + 1) * 65],
                            ems[g][:, 256 * wi: 256 * wi + 128],
                            start=True, stop=False,
                            skip_group_check=True,
                        )
                        nc.tensor.matmul(
                            pav[:, cols],
                            vsl[kw][:, h * 65: (h + 1) * 65],
                            ems[g][:, 256 * wi + 128: 256 * wi + 256],
                            start=False, stop=True,
                            skip_group_check=True,
                        )

                    outun = p_un.tile([65, 1024], f32, tag="outun",
                                      name="outun")
                    nc.scalar.copy(outun, pav)
                    den_sq = p_sc.tile([8, P], f32, tag="densq")
                    nc.sync.dma_start(den_sq, outun[64:65, :])
                    rden = p_sc.tile([8, P], f32, tag="rden")
                    nc.vector.reciprocal_approx_fast(rden, den_sq)
                    ssq = p_sc.tile([8, P], bf16, tag="ssq")
                    nc.vector.tensor_mul(
                        ssq, rden, gsq[:, (2 * h + hv) * P: (2 * h + hv + 1) * P])
                    nc.sync.dma_start(scr_d[2 * h + hv, :], ssq)
                    scale_b = p_scb.tile([64, 1024], bf16, tag="scaleb")
                    nc.sync.dma_start(
                        scale_b, scr_d[2 * h + hv, :].partition_broadcast(64))
                    nc.vector.tensor_mul(
                        y[h // 2][po: po + 64, 1024 * hv: 1024 * (hv + 1)],
                        outun[0:64, :],
                        scale_b,
                    )

                # Software pipeline: QK/exp/mask of unit n+1 is emitted before
                # AV of unit n so the PE never waits on the Scalar/DVE chain.
                # The next pair's projection is split in two and interleaved
                # between units so its RoPE and Scalar copies spread out.
                emit_proj(0)
                pending = None
                for c in range(8):
                    units = [(2 * c, 0), (2 * c, 1),
                             (2 * c + 1, 0), (2 * c + 1, 1)]
                    if c == 5:
                        for k in range(KB):
                            nc.gpsimd.dma_start(
                                wo_sb[k], wo_d[k * P: (k + 1) * P, :])
                    for ui, (h, hv) in enumerate(units):
                        if c + 1 < 8 and ui == 0:
                            emit_proj(c + 1, parts=(0,))
                        if c + 1 < 8 and ui == 2:
                            emit_proj(c + 1, parts=(1,))
                        ems = emit_qk(h, hv)
                        if pending is not None:
                            emit_av(*pending)
                        pending = (h, hv, ems)
                emit_av(*pending)
                del qk_tiles

            # ---------------- Phase 3: output projection ---------------------
            with ExitStack() as ph3:
                p_pso = ph3.enter_context(
                    tc.tile_pool(name="p_pso", bufs=4, space="PSUM"))
                p_out = ph3.enter_context(tc.tile_pool(name="p_out", bufs=3))
                for mt in range(16):
                    o_t = p_out.tile([P, DIM], f32, tag="o")
                    for nh in range(2):
                        ps = p_pso.tile([P, 512], f32, tag="pso")
                        for k in range(KB):
                            nc.tensor.matmul(
                                ps,
                                y[k][:, mt * P: (mt + 1) * P],
                                wo_sb[k][:, nh * 512: (nh + 1) * 512],
                                start=(k == 0),
                                stop=(k == KB - 1),
                            )
                        nc.scalar.copy(o_t[:, nh * 512: (nh + 1) * 512], ps)
                    eng = nc.gpsimd if (mt % 2 == 0) else nc.sync
                    eng.dma_start(out_d[mt * P: (mt + 1) * P, :], o_t)

    nc.finalize()
    return nc


def _get_program():
    if "nc" not in _PROGRAM_CACHE:
        _PROGRAM_CACHE["nc"] = _build_nc()
    return _PROGRAM_CACHE["nc"]


def _rope_tables(start: int):
    """cos/sinS tables [128, T] bf16 for shard starting at query index start.

    sinS carries the rotate-half sign so that
    rope(v) = v * cos + blockswap(v) * sinS.
    """
    pos = np.arange(start - WS, start + NQ, dtype=np.float64)
    np.clip(pos, 0, None, out=pos)
    inv = 10000.0 ** (-np.arange(0, DH, 2, dtype=np.float64) / DH)  # [32]
    d = np.arange(P) % DH
    theta = pos[None, :] * inv[d % 32][:, None]  # [128, T]
    cos = np.cos(theta)
    sinS = np.sin(theta) * np.where(d % DH < 32, -1.0, 1.0)[:, None]
    return cos.astype(BF16), sinS.astype(BF16)


def _masks(is_first_chunk: bool):
    """0/1 masks for the [back|own] 256-col slots (key-major: [key, query]).

    back: query window w vs key window w-1: valid iff key_row >= query_col.
    own:  same window: valid iff key_row <= query_col.
    mask0 zeroes the first back slot when the shard starts the sequence.
    """
    k = np.arange(P)[:, None]
    i = np.arange(P)[None, :]
    back = (k >= i).astype(np.float32)
    own = (k <= i).astype(np.float32)
    slot = np.concatenate([back, own], axis=1)          # [128, 256]
    slot0 = np.concatenate(
        [np.zeros((P, P), np.float32) if is_first_chunk else back, own], axis=1)
    mask0 = np.concatenate([slot0] + [slot] * 3, axis=1)
    masku = np.concatenate([slot] * 4, axis=1)
    return mask0.astype(BF16), masku.astype(BF16)


def kernel(x, ln_w, ln_b, w_qkv, w_gate, b_gate, w_out):
    from concourse.bass_utils import run_bass_kernel_spmd

    x = np.asarray(x, dtype=np.float32)
    ln_w = np.asarray(ln_w, dtype=np.float32)
    ln_b = np.asarray(ln_b, dtype=np.float32)
    w_qkv = np.asarray(w_qkv, dtype=np.float32)
    w_gate = np.asarray(w_gate, dtype=np.float32)
    b_gate = np.asarray(b_gate, dtype=np.float32)
    w_out = np.asarray(w_out, dtype=np.float32)

    # Fold LayerNorm affine into the projections.
    wf = ln_w[:, None] * w_qkv                     # [DIM, 3*H*DH]
    bias_qkv = ln_b @ w_qkv                        # [3*H*DH]
    wgf = ln_w[:, None] * w_gate
    bgf = (b_gate + ln_b @ w_gate).astype(np.float32)

    # Fold the attention 1/sqrt(DH) scale into the q projection (RoPE is
    # linear, so pre-scaling q is equivalent).
    wqk_f = wf[:, : 2 * DIM].copy()
    wqk_f[:, :DIM] *= QSCALE
    bqk_f = bias_qkv[: 2 * DIM].astype(np.float32).copy()
    bqk_f[:DIM] *= QSCALE
    wqk = wqk_f.astype(BF16)
    wv = wf[:, 2 * DIM:].astype(BF16)
    bqk = bqk_f.reshape(2 * DIM, 1)
    bv = bias_qkv[2 * DIM:].astype(BF16).reshape(1, DIM)
    wg = wgf.astype(BF16)
    wo = w_out.astype(BF16)

    tabs = [_rope_tables(0), _rope_tables(NQ)]
    masks = [_masks(True), _masks(False)]

    in_maps = []
    for core in range(NCORES):
        b, half = core // 2, core % 2
        start = half * NQ
        if half == 0:
            x_sh = np.concatenate(
                [np.zeros((WS, DIM), np.float32), x[b, :NQ]], axis=0
            )
        else:
            x_sh = x[b, start - WS: start + NQ]
        cos, sin = tabs[half]
        mask0, masku = masks[0 if half == 0 else 1]
        in_maps.append({
            "x": np.ascontiguousarray(x_sh),
            "wqk": wqk, "wv": wv, "wg": wg, "wo": wo,
            "bqk": bqk, "bv": bv, "bg": bgf.reshape(H, 1),
            "cos": cos, "sin": sin,
            "mask0": mask0, "masku": masku,
        })

    global _last_in_maps
    _last_in_maps = in_maps

    nc = _get_program()
    res = run_bass_kernel_spmd(nc, in_maps, list(range(NCORES)))

    out = np.empty((B, N, DIM), np.float32)
    for core in range(NCORES):
        b, half = core // 2, core % 2
        out[b, half * NQ: (half + 1) * NQ] = res.results[core]["out"]
    return out


# revision 19
# speedup vs baseline: 1.0430x; 1.0430x over previous
"""Trainium2 Bass kernel for nn_MultiHeadLocalAttention_1683627180144.

Full-input contract: kernel(**inputs) takes the complete unsharded inputs and
returns the complete output.  Internally the work is sharded 8 ways
(batch x sequence-half, with a one-window halo) and run SPMD on 8 NeuronCores
via run_bass_kernel_spmd.  Per-core differences (positions for RoPE, edge
masks) are carried entirely by the per-core input data so every core runs the
same program.

Per-core dataflow (2176 tokens = 128-token halo + 2048 queries):
  Stage A (per 128-token tile): LayerNorm (token-major, affine folded into
    weights on host) -> xn bf16; PE transposes xn into xnT (dim-major);
    v projection token-major into per-head slabs with an appended ones column
    (bias added via a ones-row matmul); gate projection head-major.
  Stage B (interleaved): q/k projection chunk pairs stream through a small
    SBUF pool; RoPE per chunk (partition-swap via small DMAs + 3 DVE muls);
    per head pair, query-window-major local attention: sim^T[key, query] in
    256-col [back|own] slots -> exp (Scalar) -> 0/1 mask mul (DVE) -> AV with
    the ones column yielding the softmax denominator in row 64 -> Scalar copy
    to SBUF -> approx-reciprocal * sigmoid-gate -> scale broadcast via DRAM
    bounce -> y^T slabs bf16.
  Phase 3: out = y^T.T @ w_out per token tile, Scalar copy, DMA to DRAM.
"""

import os
import sys

import numpy as np

for _p in ("/opt/trn_rl_repo", "/opt/pypackages"):
    if os.path.isdir(_p) and _p not in sys.path:
        sys.path.append(_p)

import ml_dtypes  # noqa: E402

import concourse.bass as bass  # noqa: E402
import concourse.bacc as bacc  # noqa: E402
import concourse.mybir as mybir  # noqa: E402
from concourse.tile import TileContext  # noqa: E402

BF16 = ml_dtypes.bfloat16

# Problem constants (hardcoded per spec).
B, N, DIM = 4, 4096, 1024
H, DH, WS = 16, 64, 128
NCORES = 8
P = 128
T = 2176          # tokens per shard incl halo window
NT = T // P       # 17 token windows
NQ = 2048         # query tokens per shard
KB = DIM // P     # 8 contraction chunks
QSCALE = DH ** -0.5

f32 = mybir.dt.float32
bf16 = mybir.dt.bfloat16

_PROGRAM_CACHE = {}


def _build_nc():
    """Build the per-core Bass program (same program on all 8 cores)."""
    nc = bacc.Bacc("TRN2")

    x_d = nc.declare_dram_parameter("x", [T, DIM], f32, isOutput=False)
    wqk_d = nc.declare_dram_parameter("wqk", [DIM, 2 * DIM], bf16, isOutput=False)
    wv_d = nc.declare_dram_parameter("wv", [DIM, DIM], bf16, isOutput=False)
    wg_d = nc.declare_dram_parameter("wg", [DIM, H], bf16, isOutput=False)
    wo_d = nc.declare_dram_parameter("wo", [DIM, DIM], bf16, isOutput=False)
    bqk_d = nc.declare_dram_parameter("bqk", [2 * DIM, 1], f32, isOutput=False)
    bv_d = nc.declare_dram_parameter("bv", [1, DIM], bf16, isOutput=False)
    bg_d = nc.declare_dram_parameter("bg", [H, 1], f32, isOutput=False)
    cos_d = nc.declare_dram_parameter("cos", [P, T], bf16, isOutput=False)
    sin_d = nc.declare_dram_parameter("sin", [P, T], bf16, isOutput=False)
    m0_d = nc.declare_dram_parameter("mask0", [P, 1024], bf16, isOutput=False)
    mu_d = nc.declare_dram_parameter("masku", [P, 1024], bf16, isOutput=False)
    out_d = nc.declare_dram_parameter("out", [NQ, DIM], f32, isOutput=True)
    # internal DRAM bounce buffer for the per-query scale broadcast
    scr_d = nc.dram_tensor("scalescr", [2 * H, 1024], bf16)

    with TileContext(nc) as tc:
        from contextlib import ExitStack

        with ExitStack() as ctx:
            consts = ctx.enter_context(tc.tile_pool(name="consts", bufs=1))
            persist = ctx.enter_context(tc.tile_pool(name="persist", bufs=1))

            # Constants.
            mask0 = consts.tile([P, 1024], bf16, tag="mask0")
            masku = consts.tile([P, 1024], bf16, tag="masku")
            nc.sync.dma_start(mask0, m0_d[:, :])
            nc.sync.dma_start(masku, mu_d[:, :])
            bg_t = consts.tile([H, 1], f32, tag="bg")
            nc.sync.dma_start(bg_t, bg_d[:, :])
            eps_t = consts.tile([P, 1], f32, tag="eps")
            nc.vector.memset(eps_t, 1e-5)
            bqk_t = consts.tile([P, 16], f32, tag="bqk")
            nc.sync.dma_start(bqk_t, bqk_d[:, :].rearrange("(m p) o -> p (m o)", p=P))
            bv_t = consts.tile([1, DIM], bf16, tag="bv")
            nc.sync.dma_start(bv_t, bv_d[:, :])
            ones1 = consts.tile([1, P], bf16, tag="ones1")
            nc.vector.memset(ones1, 1.0)
            ident = consts.tile([P, P], bf16, tag="ident")
            from concourse.masks import make_identity
            make_identity(nc, ident)
            cos_t = consts.tile([P, T], bf16, tag="cos")
            sin_t = consts.tile([P, T], bf16, tag="sin")
            nc.sync.dma_start(cos_t, cos_d[:, :])
            nc.sync.dma_start(sin_t, sin_d[:, :])

            # Persistent state.
            # xnT: dim-major normalized input; chunk k occupies cols [k*T, (k+1)*T).
            xnT = persist.tile([P, KB * T], bf16, tag="xnT", name="xnT")
            vsl = [persist.tile([P, H * 65], bf16, tag=f"v{i}", name=f"v{i}")
                   for i in range(NT)]
            gates_s = persist.tile([H, T], bf16, tag="gates_s")
            gsq = persist.tile([8, 2 * H * P], bf16, tag="gsq")
            y = [persist.tile([P, NQ], bf16, tag=f"y{i}", name=f"y{i}")
                 for i in range(KB)]

            # ---------------- Stage A: LN, PE transpose, v & gates ----------
            with ExitStack() as stA:
                p_x = stA.enter_context(tc.tile_pool(name="p_x", bufs=3))
                p_st = stA.enter_context(tc.tile_pool(name="p_st", bufs=6))
                p_xn = stA.enter_context(tc.tile_pool(name="p_xn", bufs=3))
                p_wres = stA.enter_context(tc.tile_pool(name="p_wres", bufs=1))
                p_graw = stA.enter_context(tc.tile_pool(name="p_graw", bufs=1))
                p_pt = stA.enter_context(
                    tc.tile_pool(name="p_pt", bufs=3, space="PSUM"))
                p_psv = stA.enter_context(
                    tc.tile_pool(name="p_psv", bufs=2, space="PSUM"))
                p_psg = stA.enter_context(
                    tc.tile_pool(name="p_psg", bufs=2, space="PSUM"))

                wv_sb = [p_wres.tile([P, DIM], bf16, tag=f"wv{k}", name=f"wv{k}")
                         for k in range(KB)]
                wg_sb = [p_wres.tile([P, H], bf16, tag=f"wg{k}", name=f"wg{k}")
                         for k in range(KB)]
                for k in range(KB):
                    nc.sync.dma_start(wv_sb[k], wv_d[k * P: (k + 1) * P, :])
                    nc.sync.dma_start(wg_sb[k], wg_d[k * P: (k + 1) * P, :])

                graw = p_graw.tile([H, T], f32, tag="graw")

                # gate spans end at these tiles
                gspans = {3: (0, 512), 7: (512, 1024), 11: (1024, 1536),
                          15: (1536, 2048), 16: (2048, 2176)}

                for mt in range(NT):
                    x_t = p_x.tile([P, DIM], f32, tag="x")
                    nc.gpsimd.dma_start(x_t, x_d[mt * P: (mt + 1) * P, :])
                    st = p_st.tile([P, 2, 6], f32, tag="st")
                    nc.vector.bn_stats(st[:, 0, :], x_t[:, 0:512])
                    nc.vector.bn_stats(st[:, 1, :], x_t[:, 512:1024])
                    mv = p_st.tile([P, 2], f32, tag="mv")
                    nc.vector.bn_aggr(mv, st)
                    rs = p_st.tile([P, 1], f32, tag="rs")
                    nc.scalar.activation(
                        rs, mv[:, 1:2], mybir.ActivationFunctionType.Sqrt,
                        bias=eps_t, scale=1.0,
                    )
                    nc.vector.reciprocal(rs, rs)
                    xn_t = p_xn.tile([P, DIM], bf16, tag="xn")
                    nc.vector.tensor_scalar(
                        out=xn_t, in0=x_t,
                        scalar1=mv[:, 0:1], scalar2=rs,
                        op0=mybir.AluOpType.subtract, op1=mybir.AluOpType.mult,
                    )
                    # PE transpose: xn [tok, dim] -> xnT chunks [dim, tok].
                    pt = p_pt.tile([P, KB * P], bf16, tag="pt")
                    for k in range(KB):
                        nc.tensor.transpose(
                            pt[:, k * P: (k + 1) * P],
                            xn_t[:, k * P: (k + 1) * P],
                            ident,
                        )
                    # single strided copy into the 8 chunk slots
                    nc.scalar.copy(
                        xnT.rearrange("p (kb t) -> p kb t", kb=KB)[
                            :, :, mt * P: (mt + 1) * P],
                        pt.rearrange("p (kb c) -> p kb c", kb=KB),
                    )

                    # v projection for this tile
                    nc.vector.memset(
                        vsl[mt].rearrange("p (h c) -> p h c", c=65)[:, :, 64:65],
                        1.0,
                    )
                    for half in range(2):
                        ps = p_psv.tile([P, 512], f32, tag="psv")
                        for k in range(KB):
                            nc.tensor.matmul(
                                ps,
                                xnT[:, k * T + mt * P: k * T + (mt + 1) * P],
                                wv_sb[k][:, half * 512: (half + 1) * 512],
                                start=(k == 0),
                                stop=False,
                            )
                        # bias add via ones-row matmul
                        nc.tensor.matmul(
                            ps,
                            ones1,
                            bv_t[:, half * 512: (half + 1) * 512],
                            start=False,
                            stop=True,
                        )
                        nc.scalar.copy(
                            vsl[mt].rearrange("p (h c) -> p h c", c=65)[
                                :, half * 8: (half + 1) * 8, 0:64],
                            ps.rearrange("p (h c) -> p h c", c=64),
                        )

                    if mt in gspans:
                        n0, n1 = gspans[mt]
                        psg = p_psg.tile([H, 512], f32, tag="psg")
                        for k in range(KB):
                            nc.tensor.matmul(
                                psg[:, : n1 - n0],
                                wg_sb[k],
                                xnT[:, k * T + n0: k * T + n1],
                                start=(k == 0),
                                stop=(k == KB - 1),
                            )
                        nc.scalar.activation(
                            graw[:, n0:n1], psg[:, : n1 - n0],
                            mybir.ActivationFunctionType.Identity,
                            bias=bg_t,
                        )

                # one sigmoid pass over all gates (single act-table flip)
                nc.scalar.activation(
                    gates_s, graw, mybir.ActivationFunctionType.Sigmoid)

            # squish gates into [8, 128] blocks per (head, half-of-queries)
            for h in range(H):
                for hv in range(2):
                    nc.gpsimd.dma_start(
                        gsq[:, (2 * h + hv) * P: (2 * h + hv + 1) * P],
                        gates_s[h: h + 1, P + 1024 * hv: P + 1024 * (hv + 1)],
                    )

            # ---------------- Stage B: qk proj + RoPE + attention -----------
            with ExitStack() as stB:
                p_qk = stB.enter_context(tc.tile_pool(name="p_qk", bufs=2))
                p_w = stB.enter_context(tc.tile_pool(name="p_w", bufs=2))
                p_rope = stB.enter_context(tc.tile_pool(name="p_rope", bufs=2))
                p_em = stB.enter_context(tc.tile_pool(name="p_em", bufs=6))
                p_un = stB.enter_context(tc.tile_pool(name="p_un", bufs=3))
                p_sc = stB.enter_context(tc.tile_pool(name="p_sc", bufs=2))
                p_scb = stB.enter_context(tc.tile_pool(name="p_scb", bufs=3))
                p_wo = stB.enter_context(tc.tile_pool(name="p_wo", bufs=1))
                p_ps2 = stB.enter_context(
                    tc.tile_pool(name="p_ps2", bufs=2, space="PSUM"))
                p_pav = stB.enter_context(
                    tc.tile_pool(name="p_pav", bufs=2, space="PSUM"))

                wo_sb = [p_wo.tile([P, DIM], bf16, tag=f"wo{k}", name=f"wo{k}")
                         for k in range(KB)]

                qk_tiles = {}

                def emit_proj(c, parts=(0, 1)):
                    """Project q chunk c (parts 0) and/or k chunk 8+c (part 1)
                    into the pair tile, then RoPE in place."""
                    if 0 in parts:
                        pair = p_qk.tile([P, 2, T], bf16, tag="qkpair",
                                         name=f"qkpair{c}")
                        qk_tiles[c] = pair
                    else:
                        pair = qk_tiles[c]
                    for which, ch in ((0, c), (1, 8 + c)):
                        if which not in parts:
                            continue
                        w_t = p_w.tile([P, KB, P], bf16, tag="wqk")
                        nc.sync.dma_start(
                            w_t,
                            wqk_d[:, ch * P: (ch + 1) * P].rearrange(
                                "(kb p) c -> p kb c", p=P),
                        )
                        spans = ([(P, 640), (640, 1152), (1152, 1664), (1664, T)]
                                 if which == 0 else
                                 [(0, 512), (512, 1024), (1024, 1536),
                                  (1536, 2048), (2048, T)])
                        for (n0, n1) in spans:
                            ps = p_ps2.tile([P, 1024], f32, tag="ps2")
                            for k in range(KB):
                                nc.tensor.matmul(
                                    ps[:, : n1 - n0],
                                    w_t[:, k, :],
                                    xnT[:, k * T + n0: k * T + n1],
                                    start=(k == 0),
                                    stop=(k == KB - 1),
                                )
                            nc.scalar.activation(
                                pair[:, which, n0:n1], ps[:, : n1 - n0],
                                mybir.ActivationFunctionType.Identity,
                                bias=bqk_t[:, ch: ch + 1],
                            )
                        # RoPE in place (q chunks: skip never-read halo cols).
                        r0 = P if which == 0 else 0
                        rot = p_rope.tile([P, T], bf16, tag="rot")
                        for blk in range(4):
                            src = (blk // 2) * 64 + ((blk + 1) % 2) * 32
                            nc.sync.dma_start(
                                rot[blk * 32: (blk + 1) * 32, r0:],
                                pair[src: src + 32, which, r0:],
                            )
                        qc = p_rope.tile([P, T], bf16, tag="rot", name="qc")
                        nc.vector.tensor_mul(
                            qc[:, r0:], pair[:, which, r0:], cos_t[:, r0:])
                        nc.vector.tensor_mul(
                            rot[:, r0:], rot[:, r0:], sin_t[:, r0:])
                        nc.vector.tensor_add(
                            pair[:, which, r0:], qc[:, r0:], rot[:, r0:])

                def emit_qk(h, hv):
                    """QK sim + exp + mask for one (head, query-half) unit."""
                    pair = qk_tiles[h // 2]
                    po = 64 * (h % 2)
                    qh = pair[po: po + 64, 0, :]
                    kh = pair[po: po + 64, 1, :]
                    # sim tiles A (query windows 1-4) and B (5-8), slots of
                    # 256 cols = [back(128) | own(128)] per query window.
                    ems = []
                    for g in range(2):
                        ps = p_ps2.tile([P, 1024], f32, tag="ps2", name="sim")
                        for wi in range(4):
                            w = 4 * g + wi + 1          # query window in half
                            kw = 8 * hv + w             # own key window
                            qs = (8 * hv + w) * P       # query cols
                            nc.tensor.matmul(
                                ps[:, 256 * wi: 256 * wi + 128],
                                kh[:, (kw - 1) * P: kw * P],
                                qh[:, qs: qs + P],
                                start=True, stop=True,
                            )
                            nc.tensor.matmul(
                                ps[:, 256 * wi + 128: 256 * wi + 256],
                                kh[:, kw * P: (kw + 1) * P],
                                qh[:, qs: qs + P],
                                start=True, stop=True,
                            )
                        em = p_em.tile([P, 1024], bf16, tag="em", name="em")
                        nc.scalar.activation(
                            em, ps, mybir.ActivationFunctionType.Exp)
                        mk = mask0 if (g == 0 and hv == 0) else masku
                        nc.vector.tensor_mul(em, em, mk)
                        ems.append(em)
                    return ems

                def emit_av(h, hv, ems):
                    po = 64 * (h % 2)
                    pav = p_pav.tile([65, 1024], f32, tag="pav", name="pav")
                    for w in range(1, 9):
                        g, wi = (w - 1) // 4, (w - 1) % 4
                        kw = 8 * hv + w
                        cols = slice((w - 1) * P, w * P)
                        nc.tensor.matmul(
                            pav[:, cols],
                            vsl[kw - 1][:, h * 65: (h + 1) * 65],
                            ems[g][:, 256 * wi: 256 * wi + 128],
                            start=True, stop=False,
                            skip_group_check=True,
                        )
                        nc.tensor.matmul(
                            pav[:, cols],
                            vsl[kw][:, h * 65: (h + 1) * 65],
                            ems[g][:, 256 * wi + 128: 256 * wi + 256],
                            start=False, stop=True,
                            skip_group_check=True,
                        )

                    outun = p_un.tile([65, 1024], f32, tag="outun",
                                      name="outun")
                    nc.scalar.copy(outun, pav)
                    den_sq = p_sc.tile([8, P], f32, tag="densq")
                    nc.sync.dma_start(den_sq, outun[64:65, :])
                    rden = p_sc.tile([8, P], f32, tag="rden")
                    nc.vector.reciprocal_approx_fast(rden, den_sq)
                    ssq = p_sc.tile([8, P], bf16, tag="ssq")
                    nc.vector.tensor_mul(
                        ssq, rden, gsq[:, (2 * h + hv) * P: (2 * h + hv + 1) * P])
                    nc.sync.dma_start(scr_d[2 * h + hv, :], ssq)
                    scale_b = p_scb.tile([64, 1024], bf16, tag="scaleb")
                    nc.sync.dma_start(
                        scale_b, scr_d[2 * h + hv, :].partition_broadcast(64))
                    nc.gpsimd.tensor_mul(
                        y[h // 2][po: po + 64, 1024 * hv: 1024 * (hv + 1)],
                        outun[0:64, :],
                        scale_b,
                    )

                # Software pipeline: QK/exp/mask of unit n+1 is emitted before
                # AV of unit n so the PE never waits on the Scalar/DVE chain.
                # The next pair's projection is split in two and interleaved
                # between units so its RoPE and Scalar copies spread out.
                emit_proj(0)
                from collections import deque
                pending = deque()
                for c in range(8):
                    units = [(2 * c, 0), (2 * c, 1),
                             (2 * c + 1, 0), (2 * c + 1, 1)]
                    if c == 5:
                        for k in range(KB):
                            nc.gpsimd.dma_start(
                                wo_sb[k], wo_d[k * P: (k + 1) * P, :])
                    for ui, (h, hv) in enumerate(units):
                        if c + 1 < 8 and ui == 0:
                            emit_proj(c + 1, parts=(0,))
                        if c + 1 < 8 and ui == 2:
                            emit_proj(c + 1, parts=(1,))
                        if len(pending) >= 2:
                            emit_av(*pending.popleft())
                        pending.append((h, hv, emit_qk(h, hv)))
                while pending:
                    emit_av(*pending.popleft())
                del qk_tiles

            # ---------------- Phase 3: output projection ---------------------
            with ExitStack() as ph3:
                p_pso = ph3.enter_context(
                    tc.tile_pool(name="p_pso", bufs=4, space="PSUM"))
                p_out = ph3.enter_context(tc.tile_pool(name="p_out", bufs=3))
                for mt in range(16):
                    o_t = p_out.tile([P, DIM], f32, tag="o")
                    for nh in range(2):
                        ps = p_pso.tile([P, 512], f32, tag="pso")
                        for k in range(KB):
                            nc.tensor.matmul(
                                ps,
                                y[k][:, mt * P: (mt + 1) * P],
                                wo_sb[k][:, nh * 512: (nh + 1) * 512],
                                start=(k == 0),
                                stop=(k == KB - 1),
                            )
                        nc.scalar.copy(o_t[:, nh * 512: (nh + 1) * 512], ps)
                    eng = nc.gpsimd if (mt % 2 == 0) else nc.sync
                    eng.dma_start(out_d[mt * P: (mt + 1) * P, :], o_t)

    nc.finalize()
    return nc


def _get_program():
    if "nc" not in _PROGRAM_CACHE:
        _PROGRAM_CACHE["nc"] = _build_nc()
    return _PROGRAM_CACHE["nc"]


def _rope_tables(start: int):
    """cos/sinS tables [128, T] bf16 for shard starting at query index start.

    sinS carries the rotate-half sign so that
    rope(v) = v * cos + blockswap(v) * sinS.
    """
    pos = np.arange(start - WS, start + NQ, dtype=np.float64)
    np.clip(pos, 0, None, out=pos)
    inv = 10000.0 ** (-np.arange(0, DH, 2, dtype=np.float64) / DH)  # [32]
    d = np.arange(P) % DH
    theta = pos[None, :] * inv[d % 32][:, None]  # [128, T]
    cos = np.cos(theta)
    sinS = np.sin(theta) * np.where(d % DH < 32, -1.0, 1.0)[:, None]
    return cos.astype(BF16), sinS.astype(BF16)


def _masks(is_first_chunk: bool):
    """0/1 masks for the [back|own] 256-col slots (key-major: [key, query]).

    back: query window w vs key window w-1: valid iff key_row >= query_col.
    own:  same window: valid iff key_row <= query_col.
    mask0 zeroes the first back slot when the shard starts the sequence.
    """
    k = np.arange(P)[:, None]
    i = np.arange(P)[None, :]
    back = (k >= i).astype(np.float32)
    own = (k <= i).astype(np.float32)
    slot = np.concatenate([back, own], axis=1)          # [128, 256]
    slot0 = np.concatenate(
        [np.zeros((P, P), np.float32) if is_first_chunk else back, own], axis=1)
    mask0 = np.concatenate([slot0] + [slot] * 3, axis=1)
    masku = np.concatenate([slot] * 4, axis=1)
    return mask0.astype(BF16), masku.astype(BF16)


def kernel(x, ln_w, ln_b, w_qkv, w_gate, b_gate, w_out):
    from concourse.bass_utils import run_bass_kernel_spmd

    x = np.asarray(x, dtype=np.float32)
    ln_w = np.asarray(ln_w, dtype=np.float32)
    ln_b = np.asarray(ln_b, dtype=np.float32)
    w_qkv = np.asarray(w_qkv, dtype=np.float32)
    w_gate = np.asarray(w_gate, dtype=np.float32)
    b_gate = np.asarray(b_gate, dtype=np.float32)
    w_out = np.asarray(w_out, dtype=np.float32)

    # Fold LayerNorm affine into the projections.
    wf = ln_w[:, None] * w_qkv                     # [DIM, 3*H*DH]
    bias_qkv = ln_b @ w_qkv                        # [3*H*DH]
    wgf = ln_w[:, None] * w_gate
    bgf = (b_gate + ln_b @ w_gate).astype(np.float32)

    # Fold the attention 1/sqrt(DH) scale into the q projection (RoPE is
    # linear, so pre-scaling q is equivalent).
    wqk_f = wf[:, : 2 * DIM].copy()
    wqk_f[:, :DIM] *= QSCALE
    bqk_f = bias_qkv[: 2 * DIM].astype(np.float32).copy()
    bqk_f[:DIM] *= QSCALE
    wqk = wqk_f.astype(BF16)
    wv = wf[:, 2 * DIM:].astype(BF16)
    bqk = bqk_f.reshape(2 * DIM, 1)
    bv = bias_qkv[2 * DIM:].astype(BF16).reshape(1, DIM)
    wg = wgf.astype(BF16)
    wo = w_out.astype(BF16)

    tabs = [_rope_tables(0), _rope_tables(NQ)]
    masks = [_masks(True), _masks(False)]

    in_maps = []
    for core in range(NCORES):
        b, half = core // 2, core % 2
        start = half * NQ
        if half == 0:
            x_sh = np.concatenate(
                [np.zeros((WS, DIM), np.float32), x[b, :NQ]], axis=0
            )
        else:
            x_sh = x[b, start - WS: start + NQ]
        cos, sin = tabs[half]
        mask0, masku = masks[0 if half == 0 else 1]
        in_maps.append({
            "x": np.ascontiguousarray(x_sh),
            "wqk": wqk, "wv": wv, "wg": wg, "wo": wo,
            "bqk": bqk, "bv": bv, "bg": bgf.reshape(H, 1),
            "cos": cos, "sin": sin,
            "mask0": mask0, "masku": masku,
        })

    global _last_in_maps
    _last_in_maps = in_maps

    nc = _get_program()
    res = run_bass_kernel_spmd(nc, in_maps, list(range(NCORES)))

    out = np.empty((B, N, DIM), np.float32)
    for core in range(NCORES):
        b, half = core // 2, core % 2
        out[b, half * NQ: (half + 1) * NQ] = res.results[core]["out"]
    return out
